# revision 95
# baseline (speedup 1.0000x reference)
"""Distributed Trainium2 kernel for 3-layer GraphConv GNN + global mean pool + L2 normalize.

Strategy (8 NeuronCores, SPMD), v3:
  - Nodes sharded by contiguous ranges across cores (dst-sharding of edges).
  - ONE edge schedule shared by all three layers: per core, edges sorted by
    (dst super, src chunk, dst tile, src); segments padded to 128, runs
    (super, chunk) padded to 128.  Layer 1 gathers 8-value x windows from a
    tiny [NFULL/8, 128] bf16 table; layers 2/3 gather h rows from per-chunk
    replicated tables (chunk == one int16 gather block).
  - Aggregation scatter: one-hot slot matmuls accumulate into packed PSUM
    banks (one start/stop group per bank, first/last emitted piece).
  - Dense phase fused per super.  h is exchanged in 4 chunks of decreasing
    size, each AllGather fired as soon as its supers' dense completes, so the
    collectives overlap the remaining aggregation; the last chunk's collective
    is deferred into the next layer's first super (before its last-block run)
    to avoid head-of-line blocking of the in-order gpsimd stream.
  - h2 is exchanged in fp8e4m3 (the agg3 input); per-node quantization error
    washes out in the global mean pool.  The wroot3 path keeps bf16 via
    on-core transposes.  b3 is folded into the pooled mean (counts cancel).
  - Global mean-pool accumulated in PSUM per super; partials AllGathered and
    reduced; L2 normalization replicated on all cores.

Host-side work is strictly index preprocessing / layout transforms.
"""

import os
import sys

import numpy as np

sys.path.insert(0, "/opt/trn_rl_repo")

KSKIP = set(filter(None, os.environ.get("KSKIP", "").split(",")))

import ml_dtypes  # noqa: E402

BF16 = ml_dtypes.bfloat16

# ----------------------------------------------------------------------------
# Configs
# ----------------------------------------------------------------------------

FULL_CFG = dict(N=100000, E=800000, G=64, NC=8)
DIMS = [1, 128, 256, 512]
SUPER = 8         # dst tiles per super-iteration (PSUM accumulators)
PADQ = 128        # per-(super,block,tile) segment padding quantum
BLK = 32768       # int16 index block size for gather tables
GCAP = 1024       # max indices per dma_gather call (= SWDGE ring per queue)
SL = 3072         # edges per SBUF slice (gather/one-hot staging)
NQ = 4            # SWDGE queues
SCRATCH = 16384   # dynamic dma scratch (ring = SCRATCH//16 descs per queue)
CHS = (4, 4, 3, 2)  # supers per exchange chunk; chunk k == gather block k
SERIAL_CC = False   # fire exchange collectives only after the full layer loop


def derive(cfg):
    d = dict(cfg)
    N, NC = d["N"], d["NC"]
    assert N % NC == 0
    d["NPC"] = N // NC
    d["TPC"] = (d["NPC"] + 127) // 128          # node tiles per core
    d["NPC_PAD"] = d["TPC"] * 128
    d["NFULL"] = NC * d["NPC_PAD"]
    d["NSUP"] = (d["TPC"] + SUPER - 1) // SUPER
    # chunked exchange layout: chunk k holds supers CHS[k]; chunk == block
    assert sum(CHS) == d["NSUP"]
    sup0 = np.cumsum((0,) + CHS)                # chunk super boundaries
    tile0 = np.minimum(sup0 * SUPER, d["TPC"])  # chunk tile boundaries
    d["CH_SUP0"] = sup0
    d["CH_T0"] = tile0
    d["R"] = [(int(tile0[k + 1]) - int(tile0[k])) * 128 for k in range(len(CHS))]
    d["REG"] = [NC * r for r in d["R"]]
    d["REGOFF"] = np.cumsum([0] + d["REG"])
    for reg in d["REG"]:
        assert reg <= BLK, (reg, BLK)           # one int16 block per chunk
    d["NBLK"] = len(CHS)
    return d


def _posg(cfg, src):
    """Global gather-table position for global node ids `src` (chunked core-
    major-padded layout). Returns (posg, blk, loc): posg in [0, NFULL),
    blk = chunk index, loc = chunk-local row (int16 safe)."""
    NPC = cfg["NPC"]
    t0 = cfg["CH_T0"]
    c = src // NPC
    r = src % NPC
    blk = np.searchsorted(t0[1:-1] * 128, r, side="right")
    R = np.asarray(cfg["R"], dtype=np.int64)
    regoff = np.asarray(cfg["REGOFF"], dtype=np.int64)
    loc = c * R[blk] + (r - t0[blk] * 128)
    posg = regoff[blk] + loc
    return posg, blk, loc


# ----------------------------------------------------------------------------
# Host preprocessing: edge layout + schedule
# ----------------------------------------------------------------------------

def preprocess(x, edge_index, batch, cfg):
    c = cfg
    N, E, G, NC = c["N"], c["E"], c["G"], c["NC"]
    NPC, TPC, NPC_PAD, NFULL = c["NPC"], c["TPC"], c["NPC_PAD"], c["NFULL"]
    NSUP, NBLK = c["NSUP"], c["NBLK"]

    src = np.asarray(edge_index[0], dtype=np.int64)
    dst = np.asarray(edge_index[1], dtype=np.int64)
    batch = np.asarray(batch, dtype=np.int64)

    # ---- per-core edge sets (dst-sharded), sort keys
    core_of = dst // NPC
    gdata = []
    nseg = np.zeros((NC, NSUP, NBLK, TPC), dtype=np.int64)
    for ci in range(NC):
        m = core_of == ci
        es, ed = src[m], dst[m] - ci * NPC
        tile = ed >> 7
        slot = ed & 127
        sup = tile // SUPER
        posg, blk, loc = _posg(c, es)
        order = np.lexsort((posg, tile, blk, sup))
        sup, blk, tile, slot, posg, loc = (
            a[order] for a in (sup, blk, tile, slot, posg, loc))
        np.add.at(nseg[ci], (sup, blk, tile), 1)
        gdata.append((sup, blk, tile, slot, posg, loc))

    nmax = nseg.max(axis=0)  # [NSUP, NBLK, TPC]
    npad = ((nmax + PADQ - 1) // PADQ) * PADQ
    # ensure every (sup, tile) has at least one segment so PSUM gets written
    tile_tot = npad.sum(axis=1)  # [NSUP, TPC]
    for s in range(NSUP):
        for t in range(s * SUPER, min((s + 1) * SUPER, TPC)):
            if tile_tot[s, t] == 0:
                npad[s, 0, t] = PADQ

    # run = (sup, blk), length padded to 128; segments laid out inside runs
    run_len, run_off, seg_off = {}, {}, {}
    sup_off, sup_len = {}, {}
    LT = 0
    for s in range(NSUP):
        s0 = LT
        for b in range(NBLK):
            r0 = LT
            for t in range(TPC):
                if npad[s, b, t]:
                    seg_off[(s, b, t)] = LT
                    LT += int(npad[s, b, t])
            L = LT - r0
            Lp = ((L + 127) // 128) * 128
            LT = r0 + Lp
            run_len[(s, b)] = Lp
            run_off[(s, b)] = r0
        sup_off[s] = s0
        sup_len[s] = LT - s0
    LTG = LT

    # ---- per-core edge arrays
    gidx = np.zeros((NC, LTG), dtype=np.int16)    # block-local h-row
    idx16 = np.zeros((NC, LTG), dtype=np.int16)   # xr8 row = posg//8
    woff8 = np.full((NC, LTG), -1.0, dtype=np.float32)
    slotG = np.full((NC, LTG), -1.0, dtype=np.float32)
    for ci in range(NC):
        sup, blk, tile, slot, posg, loc = gdata[ci]
        seg_ids = (sup * NBLK + blk) * TPC + tile
        bounds = np.flatnonzero(np.diff(seg_ids)) + 1
        starts = np.concatenate(([0], bounds))
        ends = np.concatenate((bounds, [len(seg_ids)]))
        for st, en in zip(starts, ends):
            s, b, t = int(sup[st]), int(blk[st]), int(tile[st])
            o = seg_off[(s, b, t)]
            n = en - st
            assert n <= npad[s, b, t], (n, npad[s, b, t])
            gidx[ci, o:o + n] = loc[st:en].astype(np.int16)
            idx16[ci, o:o + n] = (posg[st:en] // 8).astype(np.int16)
            woff8[ci, o:o + n] = (posg[st:en] % 8).astype(np.float32)
            slotG[ci, o:o + n] = slot[st:en].astype(np.float32)
            # pad entries within segment: repeat first idx (slot/woff stay -1)
            pe = o + int(npad[s, b, t])
            gidx[ci, o + n:pe] = loc[st] if n else 0
            idx16[ci, o + n:pe] = (posg[st] // 8) if n else 0

    # ---- slice-centric schedule
    SLC = SL // 128
    sched_g = []
    for s in range(NSUP):
        tiles = list(range(s * SUPER, min((s + 1) * SUPER, TPC)))
        runs = []
        for b in range(NBLK):
            L = run_len[(s, b)]
            if not L:
                continue
            ncols = L // 128
            slices = []
            for c0 in range(0, ncols, SLC):
                nc_ = min(SLC, ncols - c0)
                slices.append(dict(c0=c0, ncols=nc_,
                                   off=run_off[(s, b)] + c0 * 128,
                                   num=nc_ * 128, pieces=[]))
            runs.append(dict(b=b, off=run_off[(s, b)], num=L, slices=slices))
        run_by_b = {r["b"]: r for r in runs}
        for t in tiles:
            pieces = []
            for b in range(NBLK):
                if (s, b, t) not in seg_off:
                    continue
                o = seg_off[(s, b, t)]
                ln_tot = int(npad[s, b, t])
                lo = o - run_off[(s, b)]
                while ln_tot > 0:
                    p0 = lo % 128
                    cap = 128 if p0 == 0 else (64 if p0 == 64 else 32)
                    l = min(ln_tot, cap)
                    pieces.append((b, lo // 128, p0, l))
                    lo += l
                    ln_tot -= l
            assert pieces
            for i, (b, col, p0, l) in enumerate(pieces):
                sl = run_by_b[b]["slices"][col // SLC]
                sl["pieces"].append(dict(
                    t=t, col=col - sl["c0"], p0=p0, ln=l,
                    start=(i == 0), stop=(i == len(pieces) - 1)))
        sched_g.append(dict(tiles=tiles, runs=runs))

    # ---- idx wrap helpers
    def wrap16(a):
        L = a.shape[1]
        assert L % 16 == 0
        w = a.reshape(a.shape[0], L // 16, 16).transpose(0, 2, 1)
        return np.tile(w, (1, 8, 1)).copy()

    def wrap128(a, dtype):
        L = a.shape[1]
        assert L % 128 == 0
        return a.reshape(a.shape[0], L // 128, 128).transpose(0, 2, 1).astype(dtype).copy()

    host = {}
    host["gidx"] = wrap16(gidx)                      # [NC,128,LTG/16] i16
    host["idx16"] = wrap16(idx16)                    # [NC,128,LTG/16] i16
    host["slotG"] = wrap128(slotG, BF16)             # [NC,128,LTG/128]
    host["woff8"] = wrap128(woff8, BF16)

    # ---- x tables (layout transforms only)
    xf = np.asarray(x, dtype=np.float32).reshape(-1)
    # xpos: x values in chunked posg order, zeros in pad rows
    xpos = np.zeros((NFULL,), dtype=np.float32)
    for ci in range(NC):
        for k in range(len(CHS)):
            r0 = int(c["CH_T0"][k]) * 128
            rk = c["R"][k]
            nreal = max(0, min(NPC - r0, rk))
            if nreal > 0:
                o = int(c["REGOFF"][k]) + ci * rk
                xpos[o:o + nreal] = xf[ci * NPC + r0: ci * NPC + r0 + nreal]
    xr8 = np.zeros((NFULL // 8, 128), dtype=np.float32)
    xr8[:, 0:8] = xpos.reshape(-1, 8)
    host["xr8"] = xr8.astype(BF16)

    xloc = np.zeros((NC, 1, NPC_PAD), dtype=np.float32)
    bslot = np.full((NC, NPC_PAD), -1.0, dtype=np.float32)
    for ci in range(NC):
        xloc[ci, 0, :NPC] = xf[ci * NPC:(ci + 1) * NPC]
        bslot[ci, :NPC] = batch[ci * NPC:(ci + 1) * NPC].astype(np.float32)
    host["xloc"] = xloc.astype(BF16)
    host["bslot"] = bslot.reshape(NC, TPC, 128).transpose(0, 2, 1).astype(np.float32).copy()

    counts = np.bincount(batch, minlength=G).astype(np.float64)
    host["invcnt"] = (1.0 / np.maximum(counts, 1.0)).astype(np.float32).reshape(G, 1)

    host["onesrow"] = np.ones((1, NPC_PAD), dtype=BF16)
    host["ident"] = np.eye(128, dtype=np.float32).astype(BF16)
    host["iota128"] = np.broadcast_to(
        np.arange(128, dtype=np.float32), (128, 128)).astype(BF16).copy()
    host["iota8"] = np.broadcast_to(
        np.arange(8, dtype=np.float32), (128, 8)).astype(BF16).copy()
    host["iotaGb"] = np.broadcast_to(
        np.arange(G, dtype=np.float32), (128, G)).astype(BF16).copy()
    host["onesb"] = np.ones((1, 128), dtype=BF16)
    host["onesG"] = np.ones((1, G), dtype=BF16)

    sched = dict(sched_g=sched_g, LTG=LTG, sup_off=sup_off, sup_len=sup_len)
    return host, sched


# ----------------------------------------------------------------------------
# Graph builder
# ----------------------------------------------------------------------------

def build_graph(cfg, sched, debug=False):
    from concourse import bass, bacc, tile, mybir

    c = cfg
    G = c["G"]
    NPC_PAD, NFULL, TPC, NSUP = c["NPC_PAD"], c["NFULL"], c["TPC"], c["NSUP"]
    NBLK = c["NBLK"]
    NCH = len(CHS)
    R, REG = c["R"], c["REG"]
    CH_SUP0, CH_T0 = c["CH_SUP0"], c["CH_T0"]
    f32 = mybir.dt.float32
    bf16 = mybir.dt.bfloat16
    fp8 = mybir.dt.float8e4
    i16 = mybir.dt.int16
    AF = mybir.ActivationFunctionType
    ALU = mybir.AluOpType

    LTG = sched["LTG"]
    sup_off, sup_len = sched["sup_off"], sched["sup_len"]
    # super s -> (chunk k, is-last-super-of-chunk)
    chunk_of_sup = {}
    for k in range(NCH):
        for s in range(int(CH_SUP0[k]), int(CH_SUP0[k + 1])):
            chunk_of_sup[s] = (k, s == int(CH_SUP0[k + 1]) - 1)

    nc = bacc.Bacc("TRN2", target_bir_lowering=False, debug=debug,
                   num_devices=c["NC"], num_swdge_queues=NQ,
                   dynamic_dma_scratch_size=SCRATCH)

    def din(name, shape, dtype):
        return nc.dram_tensor(name, list(shape), dtype, kind="ExternalInput")

    p = {}
    p["xr8"] = din("xr8", (NFULL // 8, 128), bf16)
    p["xloc"] = din("xloc", (1, NPC_PAD), bf16)
    p["onesrow"] = din("onesrow", (1, NPC_PAD), bf16)
    p["w1stack"] = din("w1stack", (3, 128), bf16)
    p["wrel2"] = din("wrel2", (128, 256), f32)
    p["wroot2"] = din("wroot2", (128, 256), f32)
    p["b2"] = din("b2", (1, 256), f32)
    p["wrel3"] = din("wrel3", (256, 512), f32)
    p["wroot3"] = din("wroot3", (256, 512), f32)
    p["b3"] = din("b3", (1, 512), f32)
    p["ident"] = din("ident", (128, 128), bf16)
    p["iota128"] = din("iota128", (128, 128), bf16)
    p["iota8"] = din("iota8", (128, 8), bf16)
    p["iotaGb"] = din("iotaGb", (128, G), bf16)
    p["onesb"] = din("onesb", (1, 128), bf16)
    p["onesG"] = din("onesG", (1, G), bf16)
    p["invcnt"] = din("invcnt", (G, 1), f32)
    p["bslot"] = din("bslot", (128, TPC), f32)
    p["gidx"] = din("gidx", (128, LTG // 16), i16)
    p["idx16"] = din("idx16", (128, LTG // 16), i16)
    p["slotG"] = din("slotG", (128, LTG // 128), bf16)
    p["woff8"] = din("woff8", (128, LTG // 128), bf16)

    out_ext = nc.dram_tensor("out", [G, 512], f32, kind="ExternalOutput")

    # internal dram: per-chunk mine/full h tables; chunk k == gather block k
    h1m = [nc.dram_tensor(f"h1m_c{k}", [R[k], 128], bf16) for k in range(NCH)]
    h1f = [nc.dram_tensor(f"h1f_c{k}", [REG[k], 128], bf16, addr_space="Shared")
           for k in range(NCH)]
    h2m = [nc.dram_tensor(f"h2m_c{k}", [R[k], 256], fp8) for k in range(NCH)]
    h2f = [nc.dram_tensor(f"h2f_c{k}", [REG[k], 256], fp8, addr_space="Shared")
           for k in range(NCH)]
    pool_in = nc.dram_tensor("pool_in", [G, 512], f32)
    pool_out8 = nc.dram_tensor("pool_out8", [8 * G, 512], f32, addr_space="Shared")

    h1_blk = [(h1f[b], 0, REG[b]) for b in range(NBLK)]
    h2_blk = [(h2f[b], 0, REG[b]) for b in range(NBLK)]

    # ---------------- persistent sbuf ----------------
    h1T = nc.alloc_sbuf_tensor("h1T", [128, NPC_PAD], bf16)
    h2T0 = nc.alloc_sbuf_tensor("h2T0", [128, NPC_PAD], bf16)
    h2T1 = nc.alloc_sbuf_tensor("h2T1", [128, NPC_PAD], bf16)
    stack3 = nc.alloc_sbuf_tensor("stack3", [3, NPC_PAD], bf16)
    pooled_acc = nc.alloc_sbuf_tensor("pooled_acc", [G, 512], f32)

    ws = {}
    for name, shape, dt_ in [
        ("w1stack", (3, 128), bf16), ("ident", (128, 128), bf16),
        ("iota128", (128, 128), bf16), ("iota8", (128, 8), bf16),
        ("iotaGb", (128, G), bf16), ("onesb", (1, 128), bf16),
        ("onesG", (1, G), bf16),
        ("invcnt", (G, 1), f32), ("bslot", (128, TPC), f32),
    ]:
        ws[name] = nc.alloc_sbuf_tensor("sb_" + name, list(shape), dt_)
    wsb = {}
    for name, shape in [("wrel2", (128, 256)), ("wroot2", (128, 256)),
                        ("b2", (1, 256)), ("b3", (1, 512))]:
        wsb[name] = nc.alloc_sbuf_tensor("sbb_" + name, list(shape), bf16)
    for name in ("wrel3", "wroot3"):
        wsb[name + "_0"] = nc.alloc_sbuf_tensor("sbb_" + name + "_0", [128, 512], bf16)
        wsb[name + "_1"] = nc.alloc_sbuf_tensor("sbb_" + name + "_1", [128, 512], bf16)

    def rotq(i):
        return i % NQ

    with tile.TileContext(nc) as tc:
        # ---------------- load constants ----------------
        # constants via the scalar-engine DMA queue: the sync queue serves the
        # critical per-super idx loads and must start with super 0's.
        with tc.tile_pool(name="wtmp", bufs=2) as wtmp:
            for name in ("w1stack", "ident", "iota128", "iota8", "iotaGb",
                         "onesb", "onesG", "invcnt", "bslot"):
                nc.scalar.dma_start(ws[name].ap(), p[name].ap())
            for name in ("wrel2", "wroot2", "b2", "b3"):
                t = wtmp.tile(list(p[name].shape), f32, tag="wtmp")
                nc.scalar.dma_start(t[:], p[name].ap())
                nc.scalar.copy(wsb[name].ap(), t[:])
            for name in ("wrel3", "wroot3"):
                for k in range(2):
                    t = wtmp.tile([128, 512], f32, tag="wtmp3")
                    nc.scalar.dma_start(t[:], p[name].ap()[k * 128:(k + 1) * 128, :])
                    nc.scalar.copy(wsb[name + f"_{k}"].ap(), t[:])
            nc.scalar.dma_start(stack3.ap()[1:2, :], p["xloc"].ap())
            nc.scalar.dma_start(stack3.ap()[2:3, :], p["onesrow"].ap())

        gcall = [0]  # rolling gather-call counter for queue rotation

        def bank_flags(s_ent, pack):
            """Per-piece (start, stop) flags so each PSUM bank (holding
            128//? tiles' accumulators) has exactly one start..stop group:
            first piece emitted into a bank starts it, last stops it."""
            ti_of = {t: i for i, t in enumerate(s_ent["tiles"])}
            seq = []
            for run in s_ent["runs"]:
                for sl in run["slices"]:
                    for pc in sl["pieces"]:
                        seq.append((pc, ti_of[pc["t"]] // pack))
            first, last = {}, {}
            for k, (pc, bank) in enumerate(seq):
                if bank not in first:
                    first[bank] = k
                last[bank] = k
            flags = {}
            for k, (pc, bank) in enumerate(seq):
                flags[id(pc)] = (k == first[bank], k == last[bank])
            return flags

        def agg_supers(layer, s, s_ent, pools, pts, pack, mid_cc=None):
            """Gather + one-hot scatter for one super. layer in {1,2,3}.

            mid_cc: emitted just before the last gather block's run so the
            in-order gpsimd stream keeps working while the CC stream drains
            the previous chunks; the last block's gathers then wait only on
            this collective."""
            ipool, spool, gpool, mpool = pools
            flags = bank_flags(s_ent, pack)
            so, sn = sup_off[s], sup_len[s]
            # consolidated aux loads for the super
            if layer == 1:
                idxs = ipool.tile([128, sn // 16], i16, tag="idx")
                nc.sync.dma_start(idxs[:], p["idx16"].ap()[:, so // 16:(so + sn) // 16])
                woffs = spool.tile([128, sn // 128], bf16, tag="woff")
                nc.sync.dma_start(woffs[:], p["woff8"].ap()[:, so // 128:(so + sn) // 128])
            else:
                idxs = ipool.tile([128, sn // 16], i16, tag="idx")
                nc.sync.dma_start(idxs[:], p["gidx"].ap()[:, so // 16:(so + sn) // 16])
                woffs = None
            slots = spool.tile([128, sn // 128], bf16, tag="slot")
            nc.sync.dma_start(slots[:], p["slotG"].ap()[:, so // 128:(so + sn) // 128])

            blks = h1_blk if layer == 2 else h2_blk
            elem = {1: 128, 2: 128, 3: 256}[layer]
            gdt = fp8 if layer == 3 else bf16
            for run in s_ent["runs"]:
                b = run["b"]
                if mid_cc is not None and b == NBLK - 1:
                    mid_cc()
                    mid_cc = None
                for sl in run["slices"]:
                    off, num, C = sl["off"], sl["num"], sl["ncols"]
                    lo = off - so            # offset within super arrays
                    xg = gpool.tile([128, SL // 128, elem], gdt, tag="g")
                    if "gather" in KSKIP:
                        nc.vector.memset(xg[:, 0:C, :], 1.0)
                    for e0 in range(0, num, GCAP) if "gather" not in KSKIP else []:
                        n = min(GCAP, num - e0)
                        if layer == 1:
                            nc.gpsimd.dma_gather(
                                xg[:, e0 // 128:(e0 + n) // 128, :],
                                p["xr8"].ap(),
                                idxs[:, (lo + e0) // 16:(lo + e0 + n) // 16],
                                n, n, elem, queue_num=rotq(gcall[0]))
                        else:
                            hf, base, rows = blks[b]
                            nc.gpsimd.dma_gather(
                                xg[:, e0 // 128:(e0 + n) // 128, :],
                                hf.ap()[base:base + rows, :],
                                idxs[:, (lo + e0) // 16:(lo + e0 + n) // 16],
                                n, n, elem, queue_num=rotq(gcall[0]))
                        gcall[0] += 1
                    # S one-hot [128, C, 128]
                    S = spool.tile([128, SL // 128, 128], gdt, tag="S")
                    iota_s = ws["iota128"].ap().rearrange(
                        "p f -> p () f").broadcast_to((128, C, 128))
                    slot_b = slots[:, lo // 128:lo // 128 + C].rearrange(
                        "p c -> p c ()").broadcast_to((128, C, 128))
                    if "sbuild" not in KSKIP:
                        nc.vector.tensor_tensor(S[:, 0:C, :], iota_s, slot_b,
                                                ALU.is_equal)
                    else:
                        nc.vector.memset(S[:, 0:C, :], 0.0)
                    if layer == 1:
                        mask = mpool.tile([128, SL // 128, 8], bf16, tag="mask")
                        iota_b = ws["iota8"].ap().rearrange(
                            "p w -> p () w").broadcast_to((128, C, 8))
                        woff_b = woffs[:, lo // 128:lo // 128 + C].rearrange(
                            "p c -> p c ()").broadcast_to((128, C, 8))
                        nc.vector.tensor_tensor(mask[:, 0:C, :], iota_b, woff_b, ALU.is_equal)
                        nc.vector.tensor_tensor(mask[:, 0:C, :], mask[:, 0:C, :],
                                                xg[:, 0:C, 0:8], ALU.mult)
                        vb = mpool.tile([128, SL // 128], bf16, tag="vb")
                        with nc.allow_low_precision(
                                reason="one-hot masked window: single nonzero term"):
                            nc.vector.tensor_reduce(vb[:, 0:C], mask[:, 0:C, :],
                                                    mybir.AxisListType.X, ALU.add)
                    if "pieces" in KSKIP and sl is s_ent["runs"][0]["slices"][0]:
                        for t in s_ent["tiles"]:
                            if layer == 1:
                                nc.tensor.matmul(pts[t], S[0:1, 0, 0:1],
                                                 S[0:1, 0, :], start=True, stop=True)
                            elif layer == 2:
                                nc.tensor.matmul(pts[t], S[0:1, 0, :],
                                                 S[0:1, 0, :], start=True, stop=True)
                            else:
                                nc.tensor.matmul(pts[t], S[0:1, 0, :],
                                                 xg[0:1, 0, :], start=True, stop=True)
                    for pc in sl["pieces"] if "pieces" not in KSKIP else []:
                        t, col, p0, l = pc["t"], pc["col"], pc["p0"], pc["ln"]
                        if layer == 1:
                            lhsT = vb[p0:p0 + l, col:col + 1]
                            rhs = S[p0:p0 + l, col, :]
                        elif layer == 2:
                            lhsT = xg[p0:p0 + l, col, :]
                            rhs = S[p0:p0 + l, col, :]
                        else:
                            lhsT = S[p0:p0 + l, col, :]
                            rhs = xg[p0:p0 + l, col, :]
                        bstart, bstop = flags[id(pc)]
                        nc.tensor.matmul(pts[t], lhsT, rhs,
                                         start=bstart, stop=bstop)
            if mid_cc is not None:
                mid_cc()

        def mine_rows(s):
            """(chunk_idx, row0_in_chunk, ntiles) for super s's dense rows."""
            t0 = s * SUPER
            nt = min(SUPER, TPC - t0)
            ck = chunk_of_sup[s][0]
            return ck, (t0 - int(CH_T0[ck])) * 128, nt

        # ================= LAYER 1: agg + dense + exchange =================
        sc = nc.named_scope("l1"); sc.__enter__()
        with tc.tile_pool(name="i1", bufs=3) as ipool, \
             tc.tile_pool(name="s1", bufs=5) as spool, \
             tc.tile_pool(name="g1", bufs=5) as gpool, \
             tc.tile_pool(name="m1", bufs=3) as mpool, \
             tc.tile_pool(name="h1s", bufs=2) as hpool, \
             tc.tile_pool(name="p1", bufs=2, space="PSUM") as ppool, \
             tc.tile_pool(name="d1p", bufs=2, space="PSUM") as dpsum, \
             tc.tile_pool(name="t1p", bufs=2, space="PSUM") as tpsum:
            for s, s_ent in enumerate(sched["sched_g"]):
                pt_all = ppool.tile([1, SUPER, 128], f32, tag="ps", name=f"ps1_{s}")
                pts = {t: pt_all[0:1, ti, :]
                       for ti, t in enumerate(s_ent["tiles"])}
                # [1, 8, 128] f32 = 4KB spans two 2KB banks -> 4 tiles per bank
                if "l1agg" not in KSKIP:
                    agg_supers(1, s, s_ent, (ipool, spool, gpool, mpool), pts, 4)
                else:
                    for ti, t in enumerate(s_ent["tiles"]):
                        nc.tensor.matmul(pts[t], ws["onesb"].ap()[0:1, 0:1],
                                         ws["onesb"].ap(), start=True, stop=True)
                # evac agg1 into stack3 row 0 (one copy per bank), then dense
                ck, r0, nt = mine_rows(s)
                t0 = s_ent["tiles"][0]
                nc.scalar.copy(stack3.ap()[0:1, t0 * 128:t0 * 128 + nt * 128],
                               pt_all[0:1, 0:nt, :])
                hsup = hpool.tile([128, SUPER, 128], bf16, tag="hsup")
                for ti, t in enumerate(s_ent["tiles"]):
                    cols = slice(t * 128, (t + 1) * 128)
                    zt = dpsum.tile([128, 128], f32, tag="z")
                    nc.tensor.matmul(zt[:], stack3.ap()[:, cols],
                                     ws["w1stack"].ap(), start=True, stop=True)
                    nc.scalar.activation(hsup[:, ti, :], zt[:], AF.Relu)
                    tp = tpsum.tile([128, 128], bf16, tag="tp")
                    nc.tensor.transpose(tp[:], hsup[:, ti, :], ws["ident"].ap())
                    nc.scalar.copy(h1T.ap()[:, cols], tp[:])
                for ti in range(nt):
                    nc.sync.dma_start(
                        h1m[ck].ap()[r0 + ti * 128:r0 + (ti + 1) * 128, :],
                        hsup[:, ti, :])
                ckk, last = chunk_of_sup[s]
                if last and not SERIAL_CC and ckk < NCH - 1:
                    nc.gpsimd.collective_compute(
                        "AllGather", ALU.bypass,
                        replica_groups=[list(range(c["NC"]))],
                        ins=[h1m[ckk].ap().opt()], outs=[h1f[ckk].ap().opt()])
            if SERIAL_CC:
                for k in range(NCH):
                    nc.gpsimd.collective_compute(
                        "AllGather", ALU.bypass,
                        replica_groups=[list(range(c["NC"]))],
                        ins=[h1m[k].ap().opt()], outs=[h1f[k].ap().opt()])
        sc.__exit__(None, None, None)

        # ================= LAYER 2: agg + dense + exchange =================
        sc = nc.named_scope("l2"); sc.__enter__()
        with tc.tile_pool(name="i2", bufs=3) as ipool, \
             tc.tile_pool(name="s2", bufs=5) as spool, \
             tc.tile_pool(name="g2", bufs=5) as gpool, \
             tc.tile_pool(name="a2s", bufs=2) as apool, \
             tc.tile_pool(name="h2s", bufs=2) as hpool, \
             tc.tile_pool(name="p2", bufs=4, space="PSUM") as ppool, \
             tc.tile_pool(name="d2p", bufs=2, space="PSUM") as dpsum, \
             tc.tile_pool(name="t2p", bufs=2, space="PSUM") as tpsum:
            for s, s_ent in enumerate(sched["sched_g"]):
                ptq = [ppool.tile([128, 4, 128], f32, tag="ps", name=f"ps2_{s}_{q}")
                       for q in range(2)]
                pts = {t: ptq[ti // 4][:, ti % 4, :]
                       for ti, t in enumerate(s_ent["tiles"])}
                def _cc_h1_last():
                    nc.gpsimd.collective_compute(
                        "AllGather", ALU.bypass,
                        replica_groups=[list(range(c["NC"]))],
                        ins=[h1m[NCH - 1].ap().opt()],
                        outs=[h1f[NCH - 1].ap().opt()])
                if "l2agg" not in KSKIP:
                    agg_supers(2, s, s_ent, (ipool, spool, gpool, None), pts, 4,
                               mid_cc=_cc_h1_last if (s == 0 and not SERIAL_CC)
                               else None)
                else:
                    for ti, t in enumerate(s_ent["tiles"]):
                        nc.tensor.matmul(pts[t], ws["onesb"].ap(),
                                         ws["iota128"].ap()[0:1, :],
                                         start=True, stop=True)
                ck, r0, nt = mine_rows(s)
                a2 = apool.tile([128, SUPER, 128], bf16, tag="a2")
                for q in range(2):
                    nq_ = min(4, nt - q * 4)
                    if nq_ > 0:
                        nc.scalar.copy(a2[:, q * 4:q * 4 + nq_, :],
                                       ptq[q][:, 0:nq_, :])
                hsup = hpool.tile([128, SUPER, 256], bf16, tag="hsup")
                hsup8 = hpool.tile([128, SUPER, 256], fp8, tag="hsup8")
                for ti, t in enumerate(s_ent["tiles"]):
                    cols = slice(t * 128, (t + 1) * 128)
                    zt = dpsum.tile([128, 256], f32, tag="z")
                    nc.tensor.matmul(zt[:], a2[:, ti, :], wsb["wrel2"].ap(),
                                     start=True, stop=False)
                    nc.tensor.matmul(zt[:], h1T.ap()[:, cols], wsb["wroot2"].ap(),
                                     start=False, stop=False)
                    nc.tensor.matmul(zt[:], ws["onesb"].ap(), wsb["b2"].ap(),
                                     start=False, stop=True)
                    nc.scalar.activation(hsup[:, ti, :], zt[:], AF.Relu)
                    nc.scalar.activation(hsup8[:, ti, :], zt[:], AF.Relu)
                    for k in range(2):
                        tp = tpsum.tile([128, 128], bf16, tag="tp")
                        nc.tensor.transpose(tp[:], hsup[:, ti, k * 128:(k + 1) * 128],
                                            ws["ident"].ap())
                        dstT = h2T0 if k == 0 else h2T1
                        nc.scalar.copy(dstT.ap()[:, cols], tp[:])
                for ti in range(nt):
                    nc.sync.dma_start(
                        h2m[ck].ap()[r0 + ti * 128:r0 + (ti + 1) * 128, :],
                        hsup8[:, ti, :])
                ckk, last = chunk_of_sup[s]
                if last and not SERIAL_CC and ckk < NCH - 1:
                    nc.gpsimd.collective_compute(
                        "AllGather", ALU.bypass,
                        replica_groups=[list(range(c["NC"]))],
                        ins=[h2m[ckk].ap().opt()], outs=[h2f[ckk].ap().opt()])
            if SERIAL_CC:
                for k in range(NCH):
                    nc.gpsimd.collective_compute(
                        "AllGather", ALU.bypass,
                        replica_groups=[list(range(c["NC"]))],
                        ins=[h2m[k].ap().opt()], outs=[h2f[k].ap().opt()])
        sc.__exit__(None, None, None)

        # ================= LAYER 3: agg + dense + pool =================
        sc = nc.named_scope("l3"); sc.__enter__()
        with tc.tile_pool(name="i3", bufs=3) as ipool, \
             tc.tile_pool(name="s3", bufs=6) as spool, \
             tc.tile_pool(name="g3", bufs=6) as gpool, \
             tc.tile_pool(name="a3s", bufs=2) as apool, \
             tc.tile_pool(name="a3t", bufs=4) as atpool, \
             tc.tile_pool(name="h3s", bufs=3) as hpool, \
             tc.tile_pool(name="p3", bufs=4, space="PSUM") as ppool, \
             tc.tile_pool(name="d3p", bufs=2, space="PSUM") as dpsum, \
             tc.tile_pool(name="t3p", bufs=1, space="PSUM") as tpsum, \
             tc.tile_pool(name="plp", bufs=1, space="PSUM") as plp:
            for s, s_ent in enumerate(sched["sched_g"]):
                ptq = [ppool.tile([128, 2, 256], f32, tag="ps", name=f"ps3_{s}_{q}")
                       for q in range(4)]
                pts = {t: ptq[ti // 2][:, ti % 2, :]
                       for ti, t in enumerate(s_ent["tiles"])}
                def _cc_h2_last():
                    nc.gpsimd.collective_compute(
                        "AllGather", ALU.bypass,
                        replica_groups=[list(range(c["NC"]))],
                        ins=[h2m[NCH - 1].ap().opt()],
                        outs=[h2f[NCH - 1].ap().opt()])
                if "l3agg" not in KSKIP:
                    agg_supers(3, s, s_ent, (ipool, spool, gpool, None), pts, 2,
                               mid_cc=_cc_h2_last if (s == 0 and not SERIAL_CC)
                               else None)
                else:
                    for ti, t in enumerate(s_ent["tiles"]):
                        nc.tensor.matmul(pts[t], ws["onesb"].ap(),
                                         wsb["b2"].ap(), start=True, stop=True)
                a3 = apool.tile([128, SUPER, 256], bf16, tag="a3")
                nt = len(s_ent["tiles"])
                for q in range(4):
                    nq_ = min(2, nt - q * 2)
                    if nq_ > 0:
                        nc.scalar.copy(a3[:, q * 2:q * 2 + nq_, :],
                                       ptq[q][:, 0:nq_, :])
                pp = plp.tile([G, 512], f32, tag="pp")
                for ti, t in enumerate(s_ent["tiles"]):
                    cols = slice(t * 128, (t + 1) * 128)
                    a3T = []
                    for k in range(2):
                        tp = tpsum.tile([128, 128], bf16, tag="tp")
                        nc.tensor.transpose(tp[:], a3[:, ti, k * 128:(k + 1) * 128],
                                            ws["ident"].ap())
                        sb = atpool.tile([128, 128], bf16, tag="a3T")
                        nc.scalar.copy(sb[:], tp[:])
                        a3T.append(sb)
                    zt = dpsum.tile([128, 512], f32, tag="z")
                    nc.tensor.matmul(zt[:], a3T[0][:], wsb["wrel3_0"].ap(),
                                     start=True, stop=False)
                    nc.tensor.matmul(zt[:], a3T[1][:], wsb["wrel3_1"].ap(),
                                     start=False, stop=False)
                    nc.tensor.matmul(zt[:], h2T0.ap()[:, cols], wsb["wroot3_0"].ap(),
                                     start=False, stop=False)
                    nc.tensor.matmul(zt[:], h2T1.ap()[:, cols], wsb["wroot3_1"].ap(),
                                     start=False, stop=True)
                    ht = hpool.tile([128, 512], bf16, tag="h")
                    nc.scalar.copy(ht[:], zt[:])
                    B = hpool.tile([128, G], bf16, tag="B")
                    nc.vector.tensor_scalar(B[:], ws["iotaGb"].ap(),
                                            ws["bslot"].ap()[:, t:t + 1], None,
                                            ALU.is_equal)
                    nc.tensor.matmul(pp[:], B[:], ht[:],
                                     start=(ti == 0), stop=(ti == nt - 1))
                if s == 0:
                    nc.vector.tensor_copy(pooled_acc.ap(), pp[:])
                else:
                    nc.vector.tensor_tensor(pooled_acc.ap(), pooled_acc.ap(),
                                            pp[:], ALU.add)
        sc.__exit__(None, None, None)

        # ================= allreduce + normalize =================
        sc = nc.named_scope("final"); sc.__enter__()
        nc.sync.dma_start(pool_in.ap(), pooled_acc.ap())
        nc.gpsimd.collective_compute(
            "AllGather", ALU.bypass, replica_groups=[list(range(c["NC"]))],
            ins=[pool_in.ap().opt()], outs=[pool_out8.ap().opt()])
        with tc.tile_pool(name="fin", bufs=1) as fin, \
             tc.tile_pool(name="finp", bufs=1, space="PSUM") as finp:
            ps = fin.tile([G, 512], f32, tag="ps")
            ps8 = fin.tile([G, 8, 512], f32, tag="ps8")
            nc.sync.dma_start(
                ps8[:], pool_out8.ap().rearrange("(r g) f -> g r f", r=8))
            nc.vector.tensor_reduce(ps[:], ps8[:].rearrange("g r f -> g f r"),
                                    mybir.AxisListType.X, ALU.add)
            mean = fin.tile([G, 512], f32, tag="mean")
            nc.vector.tensor_scalar(mean[:], ps[:], ws["invcnt"].ap(), None,
                                    ALU.mult)
            # + b3 (outer(ones_G, b3)); counts cancel in the mean
            pb = finp.tile([G, 512], f32, tag="pb")
            nc.tensor.matmul(pb[:], ws["onesG"].ap(), wsb["b3"].ap(),
                             start=True, stop=True)
            nc.vector.tensor_tensor(mean[:], mean[:], pb[:], ALU.add)
            sq = fin.tile([G, 512], f32, tag="sq")
            nc.vector.tensor_tensor(sq[:], mean[:], mean[:], ALU.mult)
            ss = fin.tile([G, 1], f32, tag="ss")
            nc.vector.tensor_reduce(ss[:], sq[:], mybir.AxisListType.X, ALU.add)
            nrm = fin.tile([G, 1], f32, tag="nrm")
            nc.scalar.sqrt(nrm[:], ss[:])
            nc.vector.tensor_scalar(nrm[:], nrm[:], 1e-12, None, ALU.max)
            inv = fin.tile([G, 1], f32, tag="inv")
            nc.vector.reciprocal(inv[:], nrm[:])
            outv = fin.tile([G, 512], f32, tag="outv")
            nc.vector.tensor_scalar(outv[:], mean[:], inv[:], None, ALU.mult)
            nc.sync.dma_start(out_ext.ap(), outv[:])
        sc.__exit__(None, None, None)

    nc.compile()
    return nc


# ----------------------------------------------------------------------------
# In-map assembly
# ----------------------------------------------------------------------------

def make_in_maps(host, inputs, cfg):
    NC = cfg["NC"]
    w1stack = np.concatenate([
        np.asarray(inputs["W_rel1"], np.float32).reshape(1, 128),
        np.asarray(inputs["W_root1"], np.float32).reshape(1, 128),
        np.asarray(inputs["b_rel1"], np.float32).reshape(1, 128)], axis=0).astype(BF16)
    shared = {
        "xr8": host["xr8"],
        "onesrow": host["onesrow"],
        "w1stack": w1stack,
        "wrel2": np.asarray(inputs["W_rel2"], np.float32),
        "wroot2": np.asarray(inputs["W_root2"], np.float32),
        "b2": np.asarray(inputs["b_rel2"], np.float32).reshape(1, 256),
        "wrel3": np.asarray(inputs["W_rel3"], np.float32),
        "wroot3": np.asarray(inputs["W_root3"], np.float32),
        "b3": np.asarray(inputs["b_rel3"], np.float32).reshape(1, 512),
        "ident": host["ident"],
        "iota128": host["iota128"],
        "iota8": host["iota8"],
        "iotaGb": host["iotaGb"],
        "onesb": host["onesb"],
        "onesG": host["onesG"],
        "invcnt": host["invcnt"],
    }
    in_maps = []
    for ci in range(NC):
        m = dict(shared)
        m["xloc"] = host["xloc"][ci]
        m["bslot"] = host["bslot"][ci]
        m["gidx"] = host["gidx"][ci]
        m["idx16"] = host["idx16"][ci]
        m["slotG"] = host["slotG"][ci]
        m["woff8"] = host["woff8"][ci]
        in_maps.append(m)
    return in_maps


# ----------------------------------------------------------------------------
# Entry points
# ----------------------------------------------------------------------------

def _install_ntff_shim(so_path="/opt/axon/libaxon_pjrt.so"):
    """Provide antenv.axon_hooks (absent in this image) so that
    run_bass_kernel_spmd(trace=True) can capture NTFF profiles via the
    axon PJRT plugin's C ABI."""
    import types
    import ctypes
    import contextlib

    if "antenv.axon_hooks" in sys.modules:
        return
    try:
        lib = ctypes.CDLL(so_path)
    except OSError:
        return
    if not hasattr(lib, "axon_start_nrt_profile"):
        return
    lib.axon_start_nrt_profile.argtypes = [
        ctypes.POINTER(ctypes.c_int64), ctypes.c_size_t]
    lib.axon_start_nrt_profile.restype = ctypes.c_int64
    lib.axon_stop_nrt_profile.argtypes = [ctypes.c_char_p]
    lib.axon_stop_nrt_profile.restype = ctypes.c_int64

    @contextlib.contextmanager
    def _hook(output_dir, device_ids):
        import jax
        jax.devices()
        if device_ids:
            ids = (ctypes.c_int64 * len(device_ids))(*device_ids)
            rc = lib.axon_start_nrt_profile(ids, len(device_ids))
        else:
            rc = lib.axon_start_nrt_profile(None, 0)
        if rc != 0:
            raise RuntimeError(f"axon_start_nrt_profile rc={rc}")
        try:
            yield
        finally:
            n = lib.axon_stop_nrt_profile(str(output_dir).encode())
            if n < 0:
                raise RuntimeError(f"axon_stop_nrt_profile rc={n}")
            print(f"profile: {n} file(s) written to {output_dir}")

    mod = types.ModuleType("antenv.axon_hooks")
    mod.get_axon_ntff_profile_hook = lambda: _hook
    mod.set_axon_ntff_profile_hook = lambda h: None
    sys.modules["antenv.axon_hooks"] = mod


def run(inputs, cfg=None, sim=False, trace=False):
    cfg = derive(cfg or FULL_CFG)
    host, sched = preprocess(inputs["x"], inputs["edge_index"], inputs["batch"], cfg)
    nc = build_graph(cfg, sched, debug=sim)
    in_maps = make_in_maps(host, inputs, cfg)

    if sim:
        from concourse.bass_interp import MultiCoreSim
        s = MultiCoreSim(nc, num_cores=cfg["NC"])
        for ci in range(cfg["NC"]):
            for k, v in in_maps[ci].items():
                s.cores[ci].tensor(k)[:] = np.ascontiguousarray(v)
        s.simulate(check_with_hw=False)
        out = np.array(s.cores[0].mem_tensor("out"))
        return out, None
    else:
        if trace:
            _install_ntff_shim()
        from concourse import bass_utils
        res = bass_utils.run_bass_kernel_spmd(
            nc, in_maps, core_ids=list(range(cfg["NC"])), trace=trace)
        return np.asarray(res.results[0]["out"]), res


def kernel(**inputs) -> np.ndarray:
    out, _ = run(inputs, FULL_CFG, sim=False, trace=False)
    return out.astype(np.float32)


# revision 98
# speedup vs baseline: 1.1467x; 1.1467x over previous
"""Distributed Trainium2 kernel for 3-layer GraphConv GNN + global mean pool + L2 normalize.

Strategy (8 NeuronCores, SPMD), v3:
  - Nodes sharded by contiguous ranges across cores (dst-sharding of edges).
  - ONE edge schedule shared by all three layers: per core, edges sorted by
    (dst super, src chunk, dst tile, src); segments padded to 128, runs
    (super, chunk) padded to 128.  Layer 1 gathers 8-value x windows from a
    tiny [NFULL/8, 128] bf16 table; layers 2/3 gather h rows from per-chunk
    replicated tables (chunk == one int16 gather block).
  - Aggregation scatter: one-hot slot matmuls accumulate into packed PSUM
    banks (one start/stop group per bank, first/last emitted piece).
  - Dense phase fused per super.  h is exchanged in 4 chunks of decreasing
    size, each AllGather fired as soon as its supers' dense completes, so the
    collectives overlap the remaining aggregation; the last chunk's collective
    is deferred into the next layer's first super (before its last-block run)
    to avoid head-of-line blocking of the in-order gpsimd stream.
  - h2 is exchanged in fp8e4m3 (the agg3 input); per-node quantization error
    washes out in the global mean pool.  The wroot3 path keeps bf16 via
    on-core transposes.  b3 is folded into the pooled mean (counts cancel).
  - Global mean-pool accumulated in PSUM per super; partials AllGathered and
    reduced; L2 normalization replicated on all cores.

Host-side work is strictly index preprocessing / layout transforms.
"""

import os
import sys

import numpy as np

sys.path.insert(0, "/opt/trn_rl_repo")

KSKIP = set(filter(None, os.environ.get("KSKIP", "").split(",")))

import ml_dtypes  # noqa: E402

BF16 = ml_dtypes.bfloat16

# ----------------------------------------------------------------------------
# Configs
# ----------------------------------------------------------------------------

FULL_CFG = dict(N=100000, E=800000, G=64, NC=8)
DIMS = [1, 128, 256, 512]
SUPER = 8         # dst tiles per super-iteration (PSUM accumulators)
PADQ = 128        # per-(super,block,tile) segment padding quantum
BLK = 32768       # int16 index block size for gather tables
GCAP = 1024       # max indices per dma_gather call (= SWDGE ring per queue)
SL = 3072         # edges per SBUF slice (gather/one-hot staging)
NQ = 4            # SWDGE queues
SCRATCH = 16384   # dynamic dma scratch (ring = SCRATCH//16 descs per queue)
CHS = (4, 4, 3, 2)  # supers per exchange chunk; chunk k == gather block k
SERIAL_CC = False   # fire exchange collectives only after the full layer loop


def derive(cfg):
    d = dict(cfg)
    N, NC = d["N"], d["NC"]
    assert N % NC == 0
    d["NPC"] = N // NC
    d["TPC"] = (d["NPC"] + 127) // 128          # node tiles per core
    d["NPC_PAD"] = d["TPC"] * 128
    d["NFULL"] = NC * d["NPC_PAD"]
    d["NSUP"] = (d["TPC"] + SUPER - 1) // SUPER
    # chunked exchange layout: chunk k holds supers CHS[k]; chunk == block
    assert sum(CHS) == d["NSUP"]
    sup0 = np.cumsum((0,) + CHS)                # chunk super boundaries
    tile0 = np.minimum(sup0 * SUPER, d["TPC"])  # chunk tile boundaries
    d["CH_SUP0"] = sup0
    d["CH_T0"] = tile0
    d["R"] = [(int(tile0[k + 1]) - int(tile0[k])) * 128 for k in range(len(CHS))]
    d["REG"] = [NC * r for r in d["R"]]
    d["REGOFF"] = np.cumsum([0] + d["REG"])
    for reg in d["REG"]:
        assert reg <= BLK, (reg, BLK)           # one int16 block per chunk
    d["NBLK"] = len(CHS)
    return d


def _posg(cfg, src):
    """Global gather-table position for global node ids `src` (chunked core-
    major-padded layout). Returns (posg, blk, loc): posg in [0, NFULL),
    blk = chunk index, loc = chunk-local row (int16 safe)."""
    NPC = cfg["NPC"]
    t0 = cfg["CH_T0"]
    c = src // NPC
    r = src % NPC
    blk = np.searchsorted(t0[1:-1] * 128, r, side="right")
    R = np.asarray(cfg["R"], dtype=np.int64)
    regoff = np.asarray(cfg["REGOFF"], dtype=np.int64)
    loc = c * R[blk] + (r - t0[blk] * 128)
    posg = regoff[blk] + loc
    return posg, blk, loc


# ----------------------------------------------------------------------------
# Host preprocessing: edge layout + schedule
# ----------------------------------------------------------------------------

def preprocess(x, edge_index, batch, cfg):
    c = cfg
    N, E, G, NC = c["N"], c["E"], c["G"], c["NC"]
    NPC, TPC, NPC_PAD, NFULL = c["NPC"], c["TPC"], c["NPC_PAD"], c["NFULL"]
    NSUP, NBLK = c["NSUP"], c["NBLK"]

    src = np.asarray(edge_index[0], dtype=np.int64)
    dst = np.asarray(edge_index[1], dtype=np.int64)
    batch = np.asarray(batch, dtype=np.int64)

    # ---- per-core edge sets (dst-sharded), sort keys
    core_of = dst // NPC
    gdata = []
    nseg = np.zeros((NC, NSUP, NBLK, TPC), dtype=np.int64)
    for ci in range(NC):
        m = core_of == ci
        es, ed = src[m], dst[m] - ci * NPC
        tile = ed >> 7
        slot = ed & 127
        sup = tile // SUPER
        posg, blk, loc = _posg(c, es)
        order = np.lexsort((posg, tile, blk, sup))
        sup, blk, tile, slot, posg, loc = (
            a[order] for a in (sup, blk, tile, slot, posg, loc))
        np.add.at(nseg[ci], (sup, blk, tile), 1)
        gdata.append((sup, blk, tile, slot, posg, loc))

    nmax = nseg.max(axis=0)  # [NSUP, NBLK, TPC]
    npad = ((nmax + PADQ - 1) // PADQ) * PADQ
    # ensure every (sup, tile) has at least one segment so PSUM gets written
    tile_tot = npad.sum(axis=1)  # [NSUP, TPC]
    for s in range(NSUP):
        for t in range(s * SUPER, min((s + 1) * SUPER, TPC)):
            if tile_tot[s, t] == 0:
                npad[s, 0, t] = PADQ

    # run = (sup, blk), length padded to 128; segments laid out inside runs
    run_len, run_off, seg_off = {}, {}, {}
    sup_off, sup_len = {}, {}
    LT = 0
    for s in range(NSUP):
        s0 = LT
        for b in range(NBLK):
            r0 = LT
            for t in range(TPC):
                if npad[s, b, t]:
                    seg_off[(s, b, t)] = LT
                    LT += int(npad[s, b, t])
            L = LT - r0
            Lp = ((L + 127) // 128) * 128
            LT = r0 + Lp
            run_len[(s, b)] = Lp
            run_off[(s, b)] = r0
        sup_off[s] = s0
        sup_len[s] = LT - s0
    LTG = LT

    # ---- per-core edge arrays
    gidx = np.zeros((NC, LTG), dtype=np.int16)    # block-local h-row
    idx16 = np.zeros((NC, LTG), dtype=np.int16)   # xr8 row = posg//8
    woff8 = np.full((NC, LTG), -1.0, dtype=np.float32)
    slotG = np.full((NC, LTG), -1.0, dtype=np.float32)
    for ci in range(NC):
        sup, blk, tile, slot, posg, loc = gdata[ci]
        seg_ids = (sup * NBLK + blk) * TPC + tile
        bounds = np.flatnonzero(np.diff(seg_ids)) + 1
        starts = np.concatenate(([0], bounds))
        ends = np.concatenate((bounds, [len(seg_ids)]))
        for st, en in zip(starts, ends):
            s, b, t = int(sup[st]), int(blk[st]), int(tile[st])
            o = seg_off[(s, b, t)]
            n = en - st
            assert n <= npad[s, b, t], (n, npad[s, b, t])
            gidx[ci, o:o + n] = loc[st:en].astype(np.int16)
            idx16[ci, o:o + n] = (posg[st:en] // 8).astype(np.int16)
            woff8[ci, o:o + n] = (posg[st:en] % 8).astype(np.float32)
            slotG[ci, o:o + n] = slot[st:en].astype(np.float32)
            # pad entries within segment: repeat first idx (slot/woff stay -1)
            pe = o + int(npad[s, b, t])
            gidx[ci, o + n:pe] = loc[st] if n else 0
            idx16[ci, o + n:pe] = (posg[st] // 8) if n else 0

    # ---- slice-centric schedule
    SLC = SL // 128
    sched_g = []
    for s in range(NSUP):
        tiles = list(range(s * SUPER, min((s + 1) * SUPER, TPC)))
        runs = []
        for b in range(NBLK):
            L = run_len[(s, b)]
            if not L:
                continue
            ncols = L // 128
            slices = []
            for c0 in range(0, ncols, SLC):
                nc_ = min(SLC, ncols - c0)
                slices.append(dict(c0=c0, ncols=nc_,
                                   off=run_off[(s, b)] + c0 * 128,
                                   num=nc_ * 128, pieces=[]))
            runs.append(dict(b=b, off=run_off[(s, b)], num=L, slices=slices))
        run_by_b = {r["b"]: r for r in runs}
        for t in tiles:
            pieces = []
            for b in range(NBLK):
                if (s, b, t) not in seg_off:
                    continue
                o = seg_off[(s, b, t)]
                ln_tot = int(npad[s, b, t])
                lo = o - run_off[(s, b)]
                while ln_tot > 0:
                    p0 = lo % 128
                    cap = 128 if p0 == 0 else (64 if p0 == 64 else 32)
                    l = min(ln_tot, cap)
                    pieces.append((b, lo // 128, p0, l))
                    lo += l
                    ln_tot -= l
            assert pieces
            for i, (b, col, p0, l) in enumerate(pieces):
                sl = run_by_b[b]["slices"][col // SLC]
                sl["pieces"].append(dict(
                    t=t, col=col - sl["c0"], p0=p0, ln=l,
                    start=(i == 0), stop=(i == len(pieces) - 1)))
        sched_g.append(dict(tiles=tiles, runs=runs))

    # ---- idx wrap helpers
    def wrap16(a):
        L = a.shape[1]
        assert L % 16 == 0
        w = a.reshape(a.shape[0], L // 16, 16).transpose(0, 2, 1)
        return np.tile(w, (1, 8, 1)).copy()

    def wrap128(a, dtype):
        L = a.shape[1]
        assert L % 128 == 0
        return a.reshape(a.shape[0], L // 128, 128).transpose(0, 2, 1).astype(dtype).copy()

    host = {}
    host["gidx"] = wrap16(gidx)                      # [NC,128,LTG/16] i16
    host["idx16"] = wrap16(idx16)                    # [NC,128,LTG/16] i16
    host["slotG"] = wrap128(slotG, BF16)             # [NC,128,LTG/128]
    host["woff8"] = wrap128(woff8, BF16)

    # ---- x tables (layout transforms only)
    xf = np.asarray(x, dtype=np.float32).reshape(-1)
    # xpos: x values in chunked posg order, zeros in pad rows
    xpos = np.zeros((NFULL,), dtype=np.float32)
    for ci in range(NC):
        for k in range(len(CHS)):
            r0 = int(c["CH_T0"][k]) * 128
            rk = c["R"][k]
            nreal = max(0, min(NPC - r0, rk))
            if nreal > 0:
                o = int(c["REGOFF"][k]) + ci * rk
                xpos[o:o + nreal] = xf[ci * NPC + r0: ci * NPC + r0 + nreal]
    xr8 = np.zeros((NFULL // 8, 128), dtype=np.float32)
    xr8[:, 0:8] = xpos.reshape(-1, 8)
    host["xr8"] = xr8.astype(BF16)

    xloc = np.zeros((NC, 1, NPC_PAD), dtype=np.float32)
    bslot = np.full((NC, NPC_PAD), -1.0, dtype=np.float32)
    for ci in range(NC):
        xloc[ci, 0, :NPC] = xf[ci * NPC:(ci + 1) * NPC]
        bslot[ci, :NPC] = batch[ci * NPC:(ci + 1) * NPC].astype(np.float32)
    host["xloc"] = xloc.astype(BF16)
    host["bslot"] = bslot.reshape(NC, TPC, 128).transpose(0, 2, 1).astype(np.float32).copy()

    counts = np.bincount(batch, minlength=G).astype(np.float64)
    host["invcnt"] = (1.0 / np.maximum(counts, 1.0)).astype(np.float32).reshape(G, 1)

    host["onesrow"] = np.ones((1, NPC_PAD), dtype=BF16)
    host["ident"] = np.eye(128, dtype=np.float32).astype(BF16)
    host["iota128"] = np.broadcast_to(
        np.arange(128, dtype=np.float32), (128, 128)).astype(BF16).copy()
    host["iota8"] = np.broadcast_to(
        np.arange(8, dtype=np.float32), (128, 8)).astype(BF16).copy()
    host["iotaGb"] = np.broadcast_to(
        np.arange(G, dtype=np.float32), (128, G)).astype(BF16).copy()
    host["onesb"] = np.ones((1, 128), dtype=BF16)
    host["onesG"] = np.ones((1, G), dtype=BF16)

    sched = dict(sched_g=sched_g, LTG=LTG, sup_off=sup_off, sup_len=sup_len)
    return host, sched


# ----------------------------------------------------------------------------
# Graph builder
# ----------------------------------------------------------------------------

def build_graph(cfg, sched, debug=False):
    from concourse import bass, bacc, tile, mybir

    c = cfg
    G = c["G"]
    NPC_PAD, NFULL, TPC, NSUP = c["NPC_PAD"], c["NFULL"], c["TPC"], c["NSUP"]
    NBLK = c["NBLK"]
    NCH = len(CHS)
    R, REG = c["R"], c["REG"]
    CH_SUP0, CH_T0 = c["CH_SUP0"], c["CH_T0"]
    f32 = mybir.dt.float32
    bf16 = mybir.dt.bfloat16
    fp8 = mybir.dt.float8e4
    i16 = mybir.dt.int16
    AF = mybir.ActivationFunctionType
    ALU = mybir.AluOpType

    LTG = sched["LTG"]
    sup_off, sup_len = sched["sup_off"], sched["sup_len"]
    # super s -> (chunk k, is-last-super-of-chunk)
    chunk_of_sup = {}
    for k in range(NCH):
        for s in range(int(CH_SUP0[k]), int(CH_SUP0[k + 1])):
            chunk_of_sup[s] = (k, s == int(CH_SUP0[k + 1]) - 1)

    nc = bacc.Bacc("TRN2", target_bir_lowering=False, debug=debug,
                   num_devices=c["NC"], num_swdge_queues=NQ,
                   dynamic_dma_scratch_size=SCRATCH)

    def din(name, shape, dtype):
        return nc.dram_tensor(name, list(shape), dtype, kind="ExternalInput")

    p = {}
    p["xr8"] = din("xr8", (NFULL // 8, 128), bf16)
    p["xloc"] = din("xloc", (1, NPC_PAD), bf16)
    p["onesrow"] = din("onesrow", (1, NPC_PAD), bf16)
    p["w1stack"] = din("w1stack", (3, 128), bf16)
    p["wrel2"] = din("wrel2", (128, 256), f32)
    p["wroot2"] = din("wroot2", (128, 256), f32)
    p["b2"] = din("b2", (1, 256), f32)
    p["wrel3"] = din("wrel3", (256, 512), f32)
    p["wroot3"] = din("wroot3", (256, 512), f32)
    p["b3"] = din("b3", (1, 512), f32)
    p["ident"] = din("ident", (128, 128), bf16)
    p["iota128"] = din("iota128", (128, 128), bf16)
    p["iota8"] = din("iota8", (128, 8), bf16)
    p["iotaGb"] = din("iotaGb", (128, G), bf16)
    p["onesb"] = din("onesb", (1, 128), bf16)
    p["onesG"] = din("onesG", (1, G), bf16)
    p["invcnt"] = din("invcnt", (G, 1), f32)
    p["bslot"] = din("bslot", (128, TPC), f32)
    p["gidx"] = din("gidx", (128, LTG // 16), i16)
    p["idx16"] = din("idx16", (128, LTG // 16), i16)
    p["slotG"] = din("slotG", (128, LTG // 128), bf16)
    p["woff8"] = din("woff8", (128, LTG // 128), bf16)

    out_ext = nc.dram_tensor("out", [G, 512], f32, kind="ExternalOutput")

    # internal dram: per-chunk mine/full h tables; chunk k == gather block k
    h1m = [nc.dram_tensor(f"h1m_c{k}", [R[k], 128], bf16) for k in range(NCH)]
    h1f = [nc.dram_tensor(f"h1f_c{k}", [REG[k], 128], bf16, addr_space="Shared")
           for k in range(NCH)]
    h2m = [nc.dram_tensor(f"h2m_c{k}", [R[k], 256], fp8) for k in range(NCH)]
    h2f = [nc.dram_tensor(f"h2f_c{k}", [REG[k], 256], fp8, addr_space="Shared")
           for k in range(NCH)]
    pool_in = nc.dram_tensor("pool_in", [G, 512], f32)
    pool_out8 = nc.dram_tensor("pool_out8", [8 * G, 512], f32, addr_space="Shared")

    h1_blk = [(h1f[b], 0, REG[b]) for b in range(NBLK)]
    h2_blk = [(h2f[b], 0, REG[b]) for b in range(NBLK)]

    # ---------------- persistent sbuf ----------------
    h1T = nc.alloc_sbuf_tensor("h1T", [128, NPC_PAD], bf16)
    h2T0 = nc.alloc_sbuf_tensor("h2T0", [128, NPC_PAD], bf16)
    h2T1 = nc.alloc_sbuf_tensor("h2T1", [128, NPC_PAD], bf16)
    stack3 = nc.alloc_sbuf_tensor("stack3", [3, NPC_PAD], bf16)
    pooled_acc = nc.alloc_sbuf_tensor("pooled_acc", [G, 512], f32)

    ws = {}
    for name, shape, dt_ in [
        ("w1stack", (3, 128), bf16), ("ident", (128, 128), bf16),
        ("iota128", (128, 128), bf16), ("iota8", (128, 8), bf16),
        ("iotaGb", (128, G), bf16), ("onesb", (1, 128), bf16),
        ("onesG", (1, G), bf16),
        ("invcnt", (G, 1), f32), ("bslot", (128, TPC), f32),
    ]:
        ws[name] = nc.alloc_sbuf_tensor("sb_" + name, list(shape), dt_)
    wsb = {}
    for name, shape in [("wrel2", (128, 256)), ("wroot2", (128, 256)),
                        ("b2", (1, 256)), ("b3", (1, 512))]:
        wsb[name] = nc.alloc_sbuf_tensor("sbb_" + name, list(shape), bf16)
    for name in ("wrel3", "wroot3"):
        wsb[name + "_0"] = nc.alloc_sbuf_tensor("sbb_" + name + "_0", [128, 512], bf16)
        wsb[name + "_1"] = nc.alloc_sbuf_tensor("sbb_" + name + "_1", [128, 512], bf16)

    def rotq(i):
        return i % NQ

    with tile.TileContext(nc) as tc:
        # ---------------- load constants ----------------
        # constants via the scalar-engine DMA queue: the sync queue serves the
        # critical per-super idx loads and must start with super 0's.
        with tc.tile_pool(name="wtmp", bufs=2) as wtmp:
            for name in ("w1stack", "ident", "iota128", "iota8", "iotaGb",
                         "onesb", "onesG", "invcnt", "bslot"):
                nc.scalar.dma_start(ws[name].ap(), p[name].ap())
            for name in ("wrel2", "wroot2", "b2", "b3"):
                t = wtmp.tile(list(p[name].shape), f32, tag="wtmp")
                nc.scalar.dma_start(t[:], p[name].ap())
                nc.scalar.copy(wsb[name].ap(), t[:])
            for name in ("wrel3", "wroot3"):
                for k in range(2):
                    t = wtmp.tile([128, 512], f32, tag="wtmp3")
                    nc.scalar.dma_start(t[:], p[name].ap()[k * 128:(k + 1) * 128, :])
                    nc.scalar.copy(wsb[name + f"_{k}"].ap(), t[:])
            nc.scalar.dma_start(stack3.ap()[1:2, :], p["xloc"].ap())
            nc.scalar.dma_start(stack3.ap()[2:3, :], p["onesrow"].ap())

        gcall = [0]  # rolling gather-call counter for queue rotation

        def bank_flags(s_ent, pack):
            """Per-piece (start, stop) flags so each PSUM bank (holding
            128//? tiles' accumulators) has exactly one start..stop group:
            first piece emitted into a bank starts it, last stops it."""
            ti_of = {t: i for i, t in enumerate(s_ent["tiles"])}
            seq = []
            for run in s_ent["runs"]:
                for sl in run["slices"]:
                    for pc in sl["pieces"]:
                        seq.append((pc, ti_of[pc["t"]] // pack))
            first, last = {}, {}
            for k, (pc, bank) in enumerate(seq):
                if bank not in first:
                    first[bank] = k
                last[bank] = k
            flags = {}
            for k, (pc, bank) in enumerate(seq):
                flags[id(pc)] = (k == first[bank], k == last[bank])
            return flags

        def agg_supers(layer, s, s_ent, pools, pts, pack, mid_cc=None):
            """Gather + one-hot scatter for one super. layer in {1,2,3}.

            mid_cc: emitted just before the last gather block's run so the
            in-order gpsimd stream keeps working while the CC stream drains
            the previous chunks; the last block's gathers then wait only on
            this collective."""
            ipool, spool, gpool, mpool = pools
            flags = bank_flags(s_ent, pack)
            so, sn = sup_off[s], sup_len[s]
            # consolidated aux loads for the super
            if layer == 1:
                idxs = ipool.tile([128, sn // 16], i16, tag="idx")
                nc.sync.dma_start(idxs[:], p["idx16"].ap()[:, so // 16:(so + sn) // 16])
                woffs = spool.tile([128, sn // 128], bf16, tag="woff")
                nc.sync.dma_start(woffs[:], p["woff8"].ap()[:, so // 128:(so + sn) // 128])
            else:
                idxs = ipool.tile([128, sn // 16], i16, tag="idx")
                nc.sync.dma_start(idxs[:], p["gidx"].ap()[:, so // 16:(so + sn) // 16])
                woffs = None
            slots = spool.tile([128, sn // 128], bf16, tag="slot")
            nc.sync.dma_start(slots[:], p["slotG"].ap()[:, so // 128:(so + sn) // 128])

            blks = h1_blk if layer == 2 else h2_blk
            elem = {1: 128, 2: 128, 3: 256}[layer]
            gdt = fp8 if layer == 3 else bf16
            for run in s_ent["runs"]:
                b = run["b"]
                if mid_cc is not None and b == NBLK - 1:
                    mid_cc()
                    mid_cc = None
                for sl in run["slices"]:
                    off, num, C = sl["off"], sl["num"], sl["ncols"]
                    lo = off - so            # offset within super arrays
                    xg = gpool.tile([128, SL // 128, elem], gdt, tag="g")
                    if "gather" in KSKIP:
                        nc.vector.memset(xg[:, 0:C, :], 1.0)
                    for e0 in range(0, num, GCAP) if "gather" not in KSKIP else []:
                        n = min(GCAP, num - e0)
                        if layer == 1:
                            nc.gpsimd.dma_gather(
                                xg[:, e0 // 128:(e0 + n) // 128, :],
                                p["xr8"].ap(),
                                idxs[:, (lo + e0) // 16:(lo + e0 + n) // 16],
                                n, n, elem, queue_num=rotq(gcall[0]))
                        else:
                            hf, base, rows = blks[b]
                            nc.gpsimd.dma_gather(
                                xg[:, e0 // 128:(e0 + n) // 128, :],
                                hf.ap()[base:base + rows, :],
                                idxs[:, (lo + e0) // 16:(lo + e0 + n) // 16],
                                n, n, elem, queue_num=rotq(gcall[0]))
                        gcall[0] += 1
                    # S one-hot [128, C, 128]
                    S = spool.tile([128, SL // 128, 128], gdt, tag="S")
                    iota_s = ws["iota128"].ap().rearrange(
                        "p f -> p () f").broadcast_to((128, C, 128))
                    slot_b = slots[:, lo // 128:lo // 128 + C].rearrange(
                        "p c -> p c ()").broadcast_to((128, C, 128))
                    if "sbuild" not in KSKIP:
                        nc.vector.tensor_tensor(S[:, 0:C, :], iota_s, slot_b,
                                                ALU.is_equal)
                    else:
                        nc.vector.memset(S[:, 0:C, :], 0.0)
                    if layer == 1:
                        mask = mpool.tile([128, SL // 128, 8], bf16, tag="mask")
                        iota_b = ws["iota8"].ap().rearrange(
                            "p w -> p () w").broadcast_to((128, C, 8))
                        woff_b = woffs[:, lo // 128:lo // 128 + C].rearrange(
                            "p c -> p c ()").broadcast_to((128, C, 8))
                        nc.vector.tensor_tensor(mask[:, 0:C, :], iota_b, woff_b, ALU.is_equal)
                        nc.vector.tensor_tensor(mask[:, 0:C, :], mask[:, 0:C, :],
                                                xg[:, 0:C, 0:8], ALU.mult)
                        vb = mpool.tile([128, SL // 128], bf16, tag="vb")
                        with nc.allow_low_precision(
                                reason="one-hot masked window: single nonzero term"):
                            nc.vector.tensor_reduce(vb[:, 0:C], mask[:, 0:C, :],
                                                    mybir.AxisListType.X, ALU.add)
                    if "pieces" in KSKIP and sl is s_ent["runs"][0]["slices"][0]:
                        for t in s_ent["tiles"]:
                            if layer == 1:
                                nc.tensor.matmul(pts[t], S[0:1, 0, 0:1],
                                                 S[0:1, 0, :], start=True, stop=True)
                            elif layer == 2:
                                nc.tensor.matmul(pts[t], S[0:1, 0, :],
                                                 S[0:1, 0, :], start=True, stop=True)
                            else:
                                nc.tensor.matmul(pts[t], S[0:1, 0, :],
                                                 xg[0:1, 0, :], start=True, stop=True)
                    for pc in sl["pieces"] if "pieces" not in KSKIP else []:
                        t, col, p0, l = pc["t"], pc["col"], pc["p0"], pc["ln"]
                        if layer == 1:
                            lhsT = vb[p0:p0 + l, col:col + 1]
                            rhs = S[p0:p0 + l, col, :]
                        elif layer == 2:
                            lhsT = xg[p0:p0 + l, col, :]
                            rhs = S[p0:p0 + l, col, :]
                        else:
                            lhsT = S[p0:p0 + l, col, :]
                            rhs = xg[p0:p0 + l, col, :]
                        bstart, bstop = flags[id(pc)]
                        nc.tensor.matmul(pts[t], lhsT, rhs,
                                         start=bstart, stop=bstop)
            if mid_cc is not None:
                mid_cc()

        def mine_rows(s):
            """(chunk_idx, row0_in_chunk, ntiles) for super s's dense rows."""
            t0 = s * SUPER
            nt = min(SUPER, TPC - t0)
            ck = chunk_of_sup[s][0]
            return ck, (t0 - int(CH_T0[ck])) * 128, nt

        # ================= LAYER 1: agg + dense + exchange =================
        sc = nc.named_scope("l1"); sc.__enter__()
        with tc.tile_pool(name="i1", bufs=3) as ipool, \
             tc.tile_pool(name="s1", bufs=5) as spool, \
             tc.tile_pool(name="g1", bufs=5) as gpool, \
             tc.tile_pool(name="m1", bufs=3) as mpool, \
             tc.tile_pool(name="h1s", bufs=2) as hpool, \
             tc.tile_pool(name="p1", bufs=2, space="PSUM") as ppool, \
             tc.tile_pool(name="d1p", bufs=2, space="PSUM") as dpsum, \
             tc.tile_pool(name="t1p", bufs=2, space="PSUM") as tpsum:
            pend1 = []
            for s, s_ent in enumerate(sched["sched_g"]):
                pt_all = ppool.tile([1, SUPER, 128], f32, tag="ps", name=f"ps1_{s}")
                pts = {t: pt_all[0:1, ti, :]
                       for ti, t in enumerate(s_ent["tiles"])}
                # [1, 8, 128] f32 = 4KB spans two 2KB banks -> 4 tiles per bank
                if "l1agg" not in KSKIP:
                    agg_supers(1, s, s_ent, (ipool, spool, gpool, mpool), pts, 4)
                else:
                    for ti, t in enumerate(s_ent["tiles"]):
                        nc.tensor.matmul(pts[t], ws["onesb"].ap()[0:1, 0:1],
                                         ws["onesb"].ap(), start=True, stop=True)
                for fn in pend1:
                    fn()
                pend1 = []
                # evac agg1 into stack3 row 0 (one copy per bank), then dense
                ck, r0, nt = mine_rows(s)
                t0 = s_ent["tiles"][0]
                nc.scalar.copy(stack3.ap()[0:1, t0 * 128:t0 * 128 + nt * 128],
                               pt_all[0:1, 0:nt, :])
                hsup = hpool.tile([128, SUPER, 128], bf16, tag="hsup")
                for ti, t in enumerate(s_ent["tiles"]):
                    cols = slice(t * 128, (t + 1) * 128)
                    zt = dpsum.tile([128, 128], f32, tag="z")
                    nc.tensor.matmul(zt[:], stack3.ap()[:, cols],
                                     ws["w1stack"].ap(), start=True, stop=True)
                    nc.scalar.activation(hsup[:, ti, :], zt[:], AF.Relu)
                    tp = tpsum.tile([128, 128], bf16, tag="tp")
                    nc.tensor.transpose(tp[:], hsup[:, ti, :], ws["ident"].ap())
                    nc.scalar.copy(h1T.ap()[:, cols], tp[:])
                for ti in range(nt):
                    nc.sync.dma_start(
                        h1m[ck].ap()[r0 + ti * 128:r0 + (ti + 1) * 128, :],
                        hsup[:, ti, :])
                ckk, last = chunk_of_sup[s]
                if last and not SERIAL_CC and ckk < NCH - 1:
                    def _cc1(ckk=ckk):
                        nc.gpsimd.collective_compute(
                            "AllGather", ALU.bypass,
                            replica_groups=[list(range(c["NC"]))],
                            ins=[h1m[ckk].ap().opt()], outs=[h1f[ckk].ap().opt()])
                    if ckk == NCH - 2:
                        pend1.append(_cc1)   # emit after next super's gathers
                    else:
                        _cc1()
            if SERIAL_CC:
                for k in range(NCH):
                    nc.gpsimd.collective_compute(
                        "AllGather", ALU.bypass,
                        replica_groups=[list(range(c["NC"]))],
                        ins=[h1m[k].ap().opt()], outs=[h1f[k].ap().opt()])
        sc.__exit__(None, None, None)

        # ================= LAYER 2: agg + dense + exchange =================
        sc = nc.named_scope("l2"); sc.__enter__()
        with tc.tile_pool(name="i2", bufs=3) as ipool, \
             tc.tile_pool(name="s2", bufs=5) as spool, \
             tc.tile_pool(name="g2", bufs=5) as gpool, \
             tc.tile_pool(name="a2s", bufs=2) as apool, \
             tc.tile_pool(name="h2s", bufs=2) as hpool, \
             tc.tile_pool(name="p2", bufs=4, space="PSUM") as ppool, \
             tc.tile_pool(name="d2p", bufs=2, space="PSUM") as dpsum, \
             tc.tile_pool(name="t2p", bufs=2, space="PSUM") as tpsum:
            pend2 = []
            for s, s_ent in enumerate(sched["sched_g"]):
                ptq = [ppool.tile([128, 4, 128], f32, tag="ps", name=f"ps2_{s}_{q}")
                       for q in range(2)]
                pts = {t: ptq[ti // 4][:, ti % 4, :]
                       for ti, t in enumerate(s_ent["tiles"])}
                def _cc_h1_last():
                    nc.gpsimd.collective_compute(
                        "AllGather", ALU.bypass,
                        replica_groups=[list(range(c["NC"]))],
                        ins=[h1m[NCH - 1].ap().opt()],
                        outs=[h1f[NCH - 1].ap().opt()])
                if "l2agg" not in KSKIP:
                    agg_supers(2, s, s_ent, (ipool, spool, gpool, None), pts, 4,
                               mid_cc=_cc_h1_last if (s == 0 and not SERIAL_CC)
                               else None)
                else:
                    for ti, t in enumerate(s_ent["tiles"]):
                        nc.tensor.matmul(pts[t], ws["onesb"].ap(),
                                         ws["iota128"].ap()[0:1, :],
                                         start=True, stop=True)
                for fn in pend2:
                    fn()
                pend2 = []
                ck, r0, nt = mine_rows(s)
                a2 = apool.tile([128, SUPER, 128], bf16, tag="a2")
                for q in range(2):
                    nq_ = min(4, nt - q * 4)
                    if nq_ > 0:
                        nc.scalar.copy(a2[:, q * 4:q * 4 + nq_, :],
                                       ptq[q][:, 0:nq_, :])
                hsup = hpool.tile([128, SUPER, 256], bf16, tag="hsup")
                hsup8 = hpool.tile([128, SUPER, 256], fp8, tag="hsup8")
                for ti, t in enumerate(s_ent["tiles"]):
                    cols = slice(t * 128, (t + 1) * 128)
                    zt = dpsum.tile([128, 256], f32, tag="z")
                    nc.tensor.matmul(zt[:], a2[:, ti, :], wsb["wrel2"].ap(),
                                     start=True, stop=False)
                    nc.tensor.matmul(zt[:], h1T.ap()[:, cols], wsb["wroot2"].ap(),
                                     start=False, stop=False)
                    nc.tensor.matmul(zt[:], ws["onesb"].ap(), wsb["b2"].ap(),
                                     start=False, stop=True)
                    nc.scalar.activation(hsup[:, ti, :], zt[:], AF.Relu)
                    nc.scalar.activation(hsup8[:, ti, :], zt[:], AF.Relu)
                    for k in range(2):
                        tp = tpsum.tile([128, 128], bf16, tag="tp")
                        nc.tensor.transpose(tp[:], hsup[:, ti, k * 128:(k + 1) * 128],
                                            ws["ident"].ap())
                        dstT = h2T0 if k == 0 else h2T1
                        nc.scalar.copy(dstT.ap()[:, cols], tp[:])
                for ti in range(nt):
                    nc.sync.dma_start(
                        h2m[ck].ap()[r0 + ti * 128:r0 + (ti + 1) * 128, :],
                        hsup8[:, ti, :])
                ckk, last = chunk_of_sup[s]
                if last and not SERIAL_CC and ckk < NCH - 1:
                    def _cc2(ckk=ckk):
                        nc.gpsimd.collective_compute(
                            "AllGather", ALU.bypass,
                            replica_groups=[list(range(c["NC"]))],
                            ins=[h2m[ckk].ap().opt()], outs=[h2f[ckk].ap().opt()])
                    if ckk == NCH - 2:
                        pend2.append(_cc2)   # emit after next super's gathers
                    else:
                        _cc2()
            if SERIAL_CC:
                for k in range(NCH):
                    nc.gpsimd.collective_compute(
                        "AllGather", ALU.bypass,
                        replica_groups=[list(range(c["NC"]))],
                        ins=[h2m[k].ap().opt()], outs=[h2f[k].ap().opt()])
        sc.__exit__(None, None, None)

        # ================= LAYER 3: agg + dense + pool =================
        sc = nc.named_scope("l3"); sc.__enter__()
        with tc.tile_pool(name="i3", bufs=3) as ipool, \
             tc.tile_pool(name="s3", bufs=6) as spool, \
             tc.tile_pool(name="g3", bufs=6) as gpool, \
             tc.tile_pool(name="a3s", bufs=2) as apool, \
             tc.tile_pool(name="a3t", bufs=4) as atpool, \
             tc.tile_pool(name="h3s", bufs=3) as hpool, \
             tc.tile_pool(name="p3", bufs=4, space="PSUM") as ppool, \
             tc.tile_pool(name="d3p", bufs=2, space="PSUM") as dpsum, \
             tc.tile_pool(name="t3p", bufs=1, space="PSUM") as tpsum, \
             tc.tile_pool(name="plp", bufs=1, space="PSUM") as plp:
            for s, s_ent in enumerate(sched["sched_g"]):
                ptq = [ppool.tile([128, 2, 256], f32, tag="ps", name=f"ps3_{s}_{q}")
                       for q in range(4)]
                pts = {t: ptq[ti // 2][:, ti % 2, :]
                       for ti, t in enumerate(s_ent["tiles"])}
                def _cc_h2_last():
                    nc.gpsimd.collective_compute(
                        "AllGather", ALU.bypass,
                        replica_groups=[list(range(c["NC"]))],
                        ins=[h2m[NCH - 1].ap().opt()],
                        outs=[h2f[NCH - 1].ap().opt()])
                if "l3agg" not in KSKIP:
                    agg_supers(3, s, s_ent, (ipool, spool, gpool, None), pts, 2,
                               mid_cc=_cc_h2_last if (s == 0 and not SERIAL_CC)
                               else None)
                else:
                    for ti, t in enumerate(s_ent["tiles"]):
                        nc.tensor.matmul(pts[t], ws["onesb"].ap(),
                                         wsb["b2"].ap(), start=True, stop=True)
                a3 = apool.tile([128, SUPER, 256], bf16, tag="a3")
                nt = len(s_ent["tiles"])
                for q in range(4):
                    nq_ = min(2, nt - q * 2)
                    if nq_ > 0:
                        nc.scalar.copy(a3[:, q * 2:q * 2 + nq_, :],
                                       ptq[q][:, 0:nq_, :])
                pp = plp.tile([G, 512], f32, tag="pp")
                for ti, t in enumerate(s_ent["tiles"]):
                    cols = slice(t * 128, (t + 1) * 128)
                    a3T = []
                    for k in range(2):
                        tp = tpsum.tile([128, 128], bf16, tag="tp")
                        nc.tensor.transpose(tp[:], a3[:, ti, k * 128:(k + 1) * 128],
                                            ws["ident"].ap())
                        sb = atpool.tile([128, 128], bf16, tag="a3T")
                        nc.scalar.copy(sb[:], tp[:])
                        a3T.append(sb)
                    zt = dpsum.tile([128, 512], f32, tag="z")
                    nc.tensor.matmul(zt[:], a3T[0][:], wsb["wrel3_0"].ap(),
                                     start=True, stop=False)
                    nc.tensor.matmul(zt[:], a3T[1][:], wsb["wrel3_1"].ap(),
                                     start=False, stop=False)
                    nc.tensor.matmul(zt[:], h2T0.ap()[:, cols], wsb["wroot3_0"].ap(),
                                     start=False, stop=False)
                    nc.tensor.matmul(zt[:], h2T1.ap()[:, cols], wsb["wroot3_1"].ap(),
                                     start=False, stop=True)
                    ht = hpool.tile([128, 512], bf16, tag="h")
                    nc.scalar.copy(ht[:], zt[:])
                    B = hpool.tile([128, G], bf16, tag="B")
                    nc.vector.tensor_scalar(B[:], ws["iotaGb"].ap(),
                                            ws["bslot"].ap()[:, t:t + 1], None,
                                            ALU.is_equal)
                    nc.tensor.matmul(pp[:], B[:], ht[:],
                                     start=(ti == 0), stop=(ti == nt - 1))
                if s == 0:
                    nc.vector.tensor_copy(pooled_acc.ap(), pp[:])
                else:
                    nc.vector.tensor_tensor(pooled_acc.ap(), pooled_acc.ap(),
                                            pp[:], ALU.add)
        sc.__exit__(None, None, None)

        # ================= allreduce + normalize =================
        sc = nc.named_scope("final"); sc.__enter__()
        nc.sync.dma_start(pool_in.ap(), pooled_acc.ap())
        nc.gpsimd.collective_compute(
            "AllGather", ALU.bypass, replica_groups=[list(range(c["NC"]))],
            ins=[pool_in.ap().opt()], outs=[pool_out8.ap().opt()])
        with tc.tile_pool(name="fin", bufs=1) as fin, \
             tc.tile_pool(name="finp", bufs=1, space="PSUM") as finp:
            ps = fin.tile([G, 512], f32, tag="ps")
            ps8 = fin.tile([G, 8, 512], f32, tag="ps8")
            nc.sync.dma_start(
                ps8[:], pool_out8.ap().rearrange("(r g) f -> g r f", r=8))
            nc.vector.tensor_reduce(ps[:], ps8[:].rearrange("g r f -> g f r"),
                                    mybir.AxisListType.X, ALU.add)
            mean = fin.tile([G, 512], f32, tag="mean")
            nc.vector.tensor_scalar(mean[:], ps[:], ws["invcnt"].ap(), None,
                                    ALU.mult)
            # + b3 (outer(ones_G, b3)); counts cancel in the mean
            pb = finp.tile([G, 512], f32, tag="pb")
            nc.tensor.matmul(pb[:], ws["onesG"].ap(), wsb["b3"].ap(),
                             start=True, stop=True)
            nc.vector.tensor_tensor(mean[:], mean[:], pb[:], ALU.add)
            sq = fin.tile([G, 512], f32, tag="sq")
            nc.vector.tensor_tensor(sq[:], mean[:], mean[:], ALU.mult)
            ss = fin.tile([G, 1], f32, tag="ss")
            nc.vector.tensor_reduce(ss[:], sq[:], mybir.AxisListType.X, ALU.add)
            nrm = fin.tile([G, 1], f32, tag="nrm")
            nc.scalar.sqrt(nrm[:], ss[:])
            nc.vector.tensor_scalar(nrm[:], nrm[:], 1e-12, None, ALU.max)
            inv = fin.tile([G, 1], f32, tag="inv")
            nc.vector.reciprocal(inv[:], nrm[:])
            outv = fin.tile([G, 512], f32, tag="outv")
            nc.vector.tensor_scalar(outv[:], mean[:], inv[:], None, ALU.mult)
            nc.sync.dma_start(out_ext.ap(), outv[:])
        sc.__exit__(None, None, None)

    nc.compile()
    return nc


# ----------------------------------------------------------------------------
# In-map assembly
# ----------------------------------------------------------------------------

def make_in_maps(host, inputs, cfg):
    NC = cfg["NC"]
    w1stack = np.concatenate([
        np.asarray(inputs["W_rel1"], np.float32).reshape(1, 128),
        np.asarray(inputs["W_root1"], np.float32).reshape(1, 128),
        np.asarray(inputs["b_rel1"], np.float32).reshape(1, 128)], axis=0).astype(BF16)
    shared = {
        "xr8": host["xr8"],
        "onesrow": host["onesrow"],
        "w1stack": w1stack,
        "wrel2": np.asarray(inputs["W_rel2"], np.float32),
        "wroot2": np.asarray(inputs["W_root2"], np.float32),
        "b2": np.asarray(inputs["b_rel2"], np.float32).reshape(1, 256),
        "wrel3": np.asarray(inputs["W_rel3"], np.float32),
        "wroot3": np.asarray(inputs["W_root3"], np.float32),
        "b3": np.asarray(inputs["b_rel3"], np.float32).reshape(1, 512),
        "ident": host["ident"],
        "iota128": host["iota128"],
        "iota8": host["iota8"],
        "iotaGb": host["iotaGb"],
        "onesb": host["onesb"],
        "onesG": host["onesG"],
        "invcnt": host["invcnt"],
    }
    in_maps = []
    for ci in range(NC):
        m = dict(shared)
        m["xloc"] = host["xloc"][ci]
        m["bslot"] = host["bslot"][ci]
        m["gidx"] = host["gidx"][ci]
        m["idx16"] = host["idx16"][ci]
        m["slotG"] = host["slotG"][ci]
        m["woff8"] = host["woff8"][ci]
        in_maps.append(m)
    return in_maps


# ----------------------------------------------------------------------------
# Entry points
# ----------------------------------------------------------------------------

def _install_ntff_shim(so_path="/opt/axon/libaxon_pjrt.so"):
    """Provide antenv.axon_hooks (absent in this image) so that
    run_bass_kernel_spmd(trace=True) can capture NTFF profiles via the
    axon PJRT plugin's C ABI."""
    import types
    import ctypes
    import contextlib

    if "antenv.axon_hooks" in sys.modules:
        return
    try:
        lib = ctypes.CDLL(so_path)
    except OSError:
        return
    if not hasattr(lib, "axon_start_nrt_profile"):
        return
    lib.axon_start_nrt_profile.argtypes = [
        ctypes.POINTER(ctypes.c_int64), ctypes.c_size_t]
    lib.axon_start_nrt_profile.restype = ctypes.c_int64
    lib.axon_stop_nrt_profile.argtypes = [ctypes.c_char_p]
    lib.axon_stop_nrt_profile.restype = ctypes.c_int64

    @contextlib.contextmanager
    def _hook(output_dir, device_ids):
        import jax
        jax.devices()
        if device_ids:
            ids = (ctypes.c_int64 * len(device_ids))(*device_ids)
            rc = lib.axon_start_nrt_profile(ids, len(device_ids))
        else:
            rc = lib.axon_start_nrt_profile(None, 0)
        if rc != 0:
            raise RuntimeError(f"axon_start_nrt_profile rc={rc}")
        try:
            yield
        finally:
            n = lib.axon_stop_nrt_profile(str(output_dir).encode())
            if n < 0:
                raise RuntimeError(f"axon_stop_nrt_profile rc={n}")
            print(f"profile: {n} file(s) written to {output_dir}")

    mod = types.ModuleType("antenv.axon_hooks")
    mod.get_axon_ntff_profile_hook = lambda: _hook
    mod.set_axon_ntff_profile_hook = lambda h: None
    sys.modules["antenv.axon_hooks"] = mod


def run(inputs, cfg=None, sim=False, trace=False):
    cfg = derive(cfg or FULL_CFG)
    host, sched = preprocess(inputs["x"], inputs["edge_index"], inputs["batch"], cfg)
    nc = build_graph(cfg, sched, debug=sim)
    in_maps = make_in_maps(host, inputs, cfg)

    if sim:
        from concourse.bass_interp import MultiCoreSim
        s = MultiCoreSim(nc, num_cores=cfg["NC"])
        for ci in range(cfg["NC"]):
            for k, v in in_maps[ci].items():
                s.cores[ci].tensor(k)[:] = np.ascontiguousarray(v)
        s.simulate(check_with_hw=False)
        out = np.array(s.cores[0].mem_tensor("out"))
        return out, None
    else:
        if trace:
            _install_ntff_shim()
        from concourse import bass_utils
        res = bass_utils.run_bass_kernel_spmd(
            nc, in_maps, core_ids=list(range(cfg["NC"])), trace=trace)
        return np.asarray(res.results[0]["out"]), res


def kernel(**inputs) -> np.ndarray:
    out, _ = run(inputs, FULL_CFG, sim=False, trace=False)
    return out.astype(np.float32)
